# revision 8
# baseline (speedup 1.0000x reference)
"""NeRF MLP kernel for Trainium2 (Bass/Tile), 8-core data-parallel over rays.

v2: fp8e4 DoubleRow matmuls for every K=256 contraction (one PE pass
instead of two), bf16 for the K=30 encoder taps and the color head, and a
batched positional-encoding path.

Device layout: features on SBUF partitions, rays on the free dim.  One
chunk = one sample index s for all 512 local rays.

Key choices vs the f32r baseline:
- Weights are quantized host-side: e4m3 for the 256-wide hidden layers
  (fed to the PE in DoubleRow mode: rhs [128,2,B] packs the two K-halves,
  halving matmuls per layer), bf16 elsewhere.  Density margin to the
  relu cut is ~0.03 while the quantization shift is <4e-3, so the
  all-transparent case still yields an exactly-zero output.
- The angle path runs on groups of 8 chunks strip-packed into 128
  partitions ([128,{2,B}] tiles, 4 strips of 30 rows), so the DVE/ACT
  cost per chunk drops ~4x; a small DMA repacks sin() output to the
  [30, 8*B] layout the encoder matmuls consume.
- GpSimd (no PSUM port) takes the SBUF-only elementwise work; PSUM
  evacuations alternate ScalarE/VectorE by a static table.
- view-dir color contribution (w8v) is per-ray constant: computed once
  before the chunk loop and folded into the phase-2 tanh input, removing
  one matmul per chunk.
- Compositing identical to baseline: w_s = exp(-M_s) - exp(-(M_s+m_s))
  via two triangular matmuls (bf16 lhsT; exp(0)=1 exactly so transparent
  rays give exactly zero).
"""

import math
from contextlib import ExitStack

import numpy as np
import ml_dtypes

import concourse.bass as bass
import concourse.mybir as mybir
import concourse.tile as tile
from concourse import bacc

F32 = mybir.dt.float32
BF16 = mybir.dt.bfloat16
F8 = mybir.dt.float8e4
AF = mybir.ActivationFunctionType
OP = mybir.AluOpType
DR = mybir.MatmulPerfMode.DoubleRow

NP_BF16 = ml_dtypes.bfloat16
NP_F8 = ml_dtypes.float8_e4m3

S = 64          # samples per ray
B_FULL = 4096   # total rays
N_CORES = 8
BL = B_FULL // N_CORES  # rays per core = 512
H = 256
NEAR, FAR = 2.0, 6.0
DELTA = (FAR - NEAR) / S
L_ENC = 5
ENC = 3 * L_ENC * 2  # 30
PI = math.pi
TWO_PI = 2.0 * math.pi
MAGIC = 12582912.0  # 1.5 * 2**23, fp32 round-to-nearest trick

N_GROUPS = 8  # angle groups: 4 partition strips x 2 slots = 8 chunks each

# psum evacuation engine per (layer, half)
EVAC_ENG = {
    0: ("act", "dve"),
    1: ("dve", "act"),
    2: ("dve", "act"),
    3: ("dve", "act"),
    4: ("dve", "act"),
    5: ("dve", "dve"),
    6: ("dve", "dve"),
}


def host_constants():
    c = {}
    freqs = (2.0 ** (np.arange(L_ENC, dtype=np.float64) - 2)) * math.pi
    fturn = np.zeros((ENC, 1), dtype=np.float32)
    phase = np.zeros((ENC, 1), dtype=np.float32)
    for cc in range(3):
        for ll in range(L_ENC):
            for tt in range(2):
                j = cc * (L_ENC * 2) + ll * 2 + tt
                fturn[j, 0] = freqs[ll] / TWO_PI
                phase[j, 0] = 0.0 if tt == 0 else 0.25
    c["fturn30"] = fturn
    c["phase30"] = phase
    c["cap1e10"] = np.full((1, BL), 1.0e10, dtype=np.float32)
    c["svec64"] = (NEAR + np.arange(S, dtype=np.float32)[:, None] * DELTA).astype(
        np.float32
    )
    c["ltri"] = np.triu(np.ones((S, S))).astype(NP_BF16)
    c["ltri2"] = (np.triu(np.ones((S, S))) + np.eye(S)).astype(NP_BF16)
    c["ones31"] = np.ones((3, 1), dtype=np.float32)
    c["half641"] = np.full((S, 1), 0.5, dtype=NP_BF16)
    return c


def host_weights(inp):
    w = {}

    def kstack(m):  # [256, M] -> [128, 2, M]
        return np.ascontiguousarray(m.reshape(2, 128, m.shape[1]).transpose(1, 0, 2))

    w["w0s"] = np.asarray(inp["w0"]).astype(NP_BF16)             # [30, 256]
    for i in (1, 2, 3, 5, 6):
        w[f"w{i}s"] = kstack(np.asarray(inp[f"w{i}"])).astype(NP_F8)
    w["w4h"] = kstack(np.asarray(inp["w4"])[0:H]).astype(NP_F8)  # [128, 2, 256]
    w["w4e"] = np.asarray(inp["w4"])[H : H + ENC].astype(NP_BF16)
    w["w7f"] = kstack(np.asarray(inp["w7"])[:, 1:129]).astype(NP_F8)
    w7d = np.zeros((128, 2, 16), dtype=np.float32)
    w7d[:, :, 0:1] = kstack(np.asarray(inp["w7"])[:, 0:1])
    w["w7d"] = w7d.astype(NP_F8)
    w["w8f"] = np.asarray(inp["w8"])[0:128].astype(NP_BF16)      # [128, 3]
    w["w8v"] = np.asarray(inp["w8"])[128:131].astype(NP_BF16)    # [3, 3]
    for i in range(7):
        w[f"b{i}s"] = np.ascontiguousarray(
            np.asarray(inp[f"b{i}"]).reshape(2, 128).T
        ).astype(np.float32)                                     # [128, 2]
    w["b7f"] = np.ascontiguousarray(
        np.asarray(inp["b7"])[1:129, None]).astype(np.float32)
    w["b7d64"] = np.full((S, 1), np.asarray(inp["b7"])[0], dtype=np.float32)
    w["b8vec3"] = np.ascontiguousarray(
        np.asarray(inp["b8"])[:, None]).astype(np.float32)
    return w


def input_specs():
    """name -> (shape, dtype) for every ExternalInput."""
    specs = {
        "xT": ((6, BL), F32),
        "off": ((S, BL), F32),
        "w0s": ((30, 256), BF16),
        "w4h": ((128, 2, 256), F8),
        "w4e": ((30, 256), BF16),
        "w7f": ((128, 2, 128), F8),
        "w7d": ((128, 2, 16), F8),
        "w8f": ((128, 3), BF16),
        "w8v": ((3, 3), BF16),
        "b7f": ((128, 1), F32),
        "b7d64": ((S, 1), F32),
        "b8vec3": ((3, 1), F32),
        "cap1e10": ((1, BL), F32),
        "fturn30": ((ENC, 1), F32),
        "phase30": ((ENC, 1), F32),
        "svec64": ((S, 1), F32),
        "ltri": ((S, S), BF16),
        "ltri2": ((S, S), BF16),
        "ones31": ((3, 1), F32),
        "half641": ((S, 1), BF16),
    }
    for i in (1, 2, 3, 5, 6):
        specs[f"w{i}s"] = ((128, 2, 256), F8)
    for i in range(7):
        specs[f"b{i}s"] = ((128, 2), F32)
    return specs


CONST_NAMES = tuple(sorted(input_specs().keys() - {"xT", "off"}))


def bcast_rows(ap, reps, cols):
    """Source AP repeating each row of `ap` `reps` times (for DMA)."""
    rows = ap.shape[0]
    return bass.AP(
        tensor=ap.tensor,
        offset=ap.offset,
        ap=[[ap.ap[0][0], rows], [0, reps], [1, cols]],
    )


def rep_free(ap, reps, cols):
    """[P, cols] -> [P, reps, cols] with 0-stride free-dim repetition."""
    return bass.AP(
        tensor=ap.tensor,
        offset=ap.offset,
        ap=[list(ap.ap[0]), [0, reps], [1, cols]],
    )


def build_nerf(tc, ctx, out_ap, a):
    nc = tc.nc
    B = BL

    consts = ctx.enter_context(tc.tile_pool(name="consts", bufs=1))
    pre = ctx.enter_context(tc.tile_pool(name="pre", bufs=1))
    agrp = ctx.enter_context(tc.tile_pool(name="agrp", bufs=2))
    work = ctx.enter_context(tc.tile_pool(name="work", bufs=3))
    psA = ctx.enter_context(tc.tile_pool(name="psA", bufs=3, space="PSUM"))
    psB = ctx.enter_context(tc.tile_pool(name="psB", bufs=2, space="PSUM"))

    # ---- constants / weights ----
    sb = {}
    specs = input_specs()
    for name in CONST_NAMES:
        t = consts.tile(list(specs[name][0]), specs[name][1], name=name, tag=name)
        nc.sync.dma_start(out=t, in_=a[name])
        sb[name] = t

    dt3 = pre.tile([3, B], F32, name="dt3", tag="dt3")
    nc.sync.dma_start(out=dt3, in_=a["xT"][3:6])
    off = pre.tile([S, B], F32, name="off", tag="off")
    nc.sync.dma_start(out=off, in_=a["off"])

    # per-ray encoding constants: angle/2pi = DF*z + AO
    D30 = pre.tile([ENC, B], F32, name="D30", tag="D30")
    nc.sync.dma_start(out=D30, in_=bcast_rows(a["xT"][3:6], 2 * L_ENC, B))
    O30 = pre.tile([ENC, B], F32, name="O30", tag="O30")
    nc.sync.dma_start(out=O30, in_=bcast_rows(a["xT"][0:3], 2 * L_ENC, B))
    DF = pre.tile([ENC, B], F32, name="DF", tag="DF")
    nc.vector.tensor_scalar(out=DF, in0=D30, scalar1=sb["fturn30"],
                            scalar2=None, op0=OP.mult)
    AO = pre.tile([ENC, B], F32, name="AO", tag="AO")
    nc.vector.tensor_scalar(out=AO, in0=O30, scalar1=sb["fturn30"],
                            scalar2=sb["phase30"], op0=OP.mult, op1=OP.add)

    # strip-replicated DF/AO for the batched angle path
    DF2 = pre.tile([128, B], F32, name="DF2", tag="DF2")
    AO2 = pre.tile([128, B], F32, name="AO2", tag="AO2")
    for j in range(4):
        nc.sync.dma_start(out=DF2[32 * j : 32 * j + ENC], in_=DF)
        nc.sync.dma_start(out=AO2[32 * j : 32 * j + ENC], in_=AO)

    # Z[s, b] = NEAR + (s + off) * DELTA
    Z = pre.tile([S, B], F32, name="Z", tag="Z")
    nc.vector.tensor_scalar(out=Z, in0=off, scalar1=DELTA, scalar2=sb["svec64"],
                            op0=OP.mult, op1=OP.add)

    # |d| and view_dir
    sq3 = pre.tile([3, B], F32, name="sq3", tag="sq3")
    nc.vector.tensor_mul(sq3, dt3, dt3)
    ps_nd = psA.tile([128, 2, B], F32, name="mmA", tag="mmA")[0:1, 0, :]
    nc.tensor.matmul(ps_nd, sb["ones31"], sq3, start=True, stop=True)
    nd = pre.tile([1, B], F32, name="nd", tag="nd")
    nc.scalar.activation(out=nd, in_=ps_nd, func=AF.Sqrt)
    inv_nd = pre.tile([1, B], F32, name="inv_nd", tag="inv_nd")
    nc.vector.reciprocal(out=inv_nd, in_=nd)
    inv3 = pre.tile([3, B], F32, name="inv3", tag="inv3")
    nc.gpsimd.partition_broadcast(inv3, inv_nd)
    v3 = pre.tile([3, B], BF16, name="v3", tag="v3")
    nc.vector.tensor_mul(v3, dt3, inv3)

    # per-ray color offset cv_c = (w8v.T v3)_c + b8_c, broadcast to 64 rows
    ps_cv = psA.tile([128, 2, B], F32, name="mmA", tag="mmA")[0:3, 0, :]
    nc.tensor.matmul(ps_cv, sb["w8v"], v3, start=True, stop=True)
    CVS = pre.tile([3, B], F32, name="CVS", tag="CVS")
    nc.vector.tensor_scalar(out=CVS, in0=ps_cv, scalar1=sb["b8vec3"],
                            scalar2=None, op0=OP.add)
    CVB = []
    for c in range(3):
        t = pre.tile([S, B], F32, name=f"CVB{c}", tag=f"CVB{c}")
        nc.sync.dma_start(out=t, in_=bcast_rows(CVS[c : c + 1], S, B))
        CVB.append(t)

    # dists
    nd64 = pre.tile([S, B], F32, name="nd64", tag="nd64")
    nc.gpsimd.partition_broadcast(nd64, nd)
    ZN = pre.tile([S, B], F32, name="ZN", tag="ZN")
    nc.vector.tensor_mul(ZN, Z, nd64)
    ZNs = pre.tile([S, B], F32, name="ZNs", tag="ZNs")
    nc.sync.dma_start(out=ZNs[0 : S - 1], in_=ZN[1:S])
    nc.sync.dma_start(out=ZNs[S - 1 : S], in_=a["cap1e10"])
    dists = pre.tile([S, B], F32, name="dists", tag="dists")
    nc.vector.tensor_sub(dists, ZNs, ZN)

    # raw density + color-tanh rows, written during the chunk loop
    DTH = pre.tile([S, 4, B], F32, name="DTH", tag="DTH")

    def emit_angle_group(g):
        """Angle path for chunks {16j + 2g + l}: strip-packed [128,{2,B}]."""
        zg = agrp.tile([128, 2, B], F32, name="zg", tag="zg")
        for j in range(4):
            for l in range(2):
                s = 16 * j + 2 * g + l
                nc.sync.dma_start(
                    out=zg[32 * j : 32 * j + ENC, l, :],
                    in_=bcast_rows(Z[s : s + 1], ENC, B),
                )
        u = agrp.tile([128, 2, B], F32, name="u", tag="u")
        nc.gpsimd.tensor_mul(u, zg, rep_free(DF2, 2, B))
        nc.gpsimd.tensor_add(u, u, rep_free(AO2, 2, B))
        kk = agrp.tile([128, 2, B], F32, name="kk", tag="kk")
        nc.vector.tensor_scalar(out=kk, in0=u, scalar1=MAGIC, scalar2=MAGIC,
                                op0=OP.add, op1=OP.subtract)
        nc.vector.tensor_sub(u, u, kk)
        encs = agrp.tile([128, 2, B], BF16, name="encs", tag="encs")
        nc.scalar.activation(out=encs, in_=u, func=AF.Sin, scale=TWO_PI)
        encg = agrp.tile([ENC, 8, B], BF16, name="encg", tag="encg")
        for j in range(4):
            nc.sync.dma_start(
                out=encg[:, 2 * j : 2 * j + 2, :],
                in_=encs[32 * j : 32 * j + ENC, :, :],
            )
        return encg

    def evac(p, h, bname, engines):
        for m, eng in enumerate(engines):
            bias = sb[bname][:, m : m + 1]
            if eng == "act":
                nc.scalar.activation(out=h[:, m, :], in_=p[:, m, :],
                                     func=AF.Relu, bias=bias)
            else:
                nc.vector.tensor_scalar(
                    out=h[:, m, :], in0=p[:, m, :], scalar1=bias, scalar2=0.0,
                    op0=OP.add, op1=OP.max,
                )

    def chunk_mlp(s, enc_s):
        # L0 (bf16, K=30)
        p = psA.tile([128, 2, B], F32, name="mmA", tag="mmA")
        for m in (0, 1):
            nc.tensor.matmul(p[:, m, :], sb["w0s"][:, m * 128 : (m + 1) * 128],
                             enc_s, start=True, stop=True)
        h0 = work.tile([128, 2, B], F8, name="h0", tag="h")
        evac(p, h0, "b0s", EVAC_ENG[0])
        hp = h0

        # L1-3 (fp8 DoubleRow)
        for li in (1, 2, 3):
            p = psA.tile([128, 2, B], F32, name="mmA", tag="mmA")
            w = sb[f"w{li}s"]
            for m in (0, 1):
                nc.tensor.matmul(p[:, m, :], w[:, :, m * 128 : (m + 1) * 128],
                                 hp, start=True, stop=True, perf_mode=DR)
            h = work.tile([128, 2, B], F8, name=f"h{li}", tag="h")
            evac(p, h, f"b{li}s", EVAC_ENG[li])
            hp = h

        # L4: DR h-part + bf16 enc skip
        p = psA.tile([128, 2, B], F32, name="mmA", tag="mmA")
        for m in (0, 1):
            nc.tensor.matmul(p[:, m, :], sb["w4h"][:, :, m * 128 : (m + 1) * 128],
                             hp, start=True, stop=False, perf_mode=DR,
                             skip_group_check=True)
            nc.tensor.matmul(p[:, m, :], sb["w4e"][:, m * 128 : (m + 1) * 128],
                             enc_s, start=False, stop=True,
                             skip_group_check=True)
        h4 = work.tile([128, 2, B], F8, name="h4", tag="h")
        evac(p, h4, "b4s", EVAC_ENG[4])
        hp = h4

        for li in (5, 6):
            p = psA.tile([128, 2, B], F32, name="mmA", tag="mmA")
            w = sb[f"w{li}s"]
            for m in (0, 1):
                nc.tensor.matmul(p[:, m, :], w[:, :, m * 128 : (m + 1) * 128],
                                 hp, start=True, stop=True, perf_mode=DR)
            h = work.tile([128, 2, B], F8, name=f"h{li}", tag="h")
            evac(p, h, f"b{li}s", EVAC_ENG[li])
            hp = h

        # L7 features -> relu -> F1 (bf16)
        p7 = psA.tile([128, 2, B], F32, name="mmA", tag="mmA")
        nc.tensor.matmul(p7[:, 0, :], sb["w7f"], hp, start=True, stop=True,
                         perf_mode=DR)
        F1 = work.tile([128, B], BF16, name="F1", tag="F1")
        nc.scalar.activation(out=F1, in_=p7[:, 0, :], func=AF.Relu,
                             bias=sb["b7f"])

        # density (row 0) + color pre-activations (rows 32-34) in one bank
        pdt = psB.tile([128, B], F32, name="pdt", tag="pdt")
        nc.tensor.matmul(pdt[0:16, :], sb["w7d"], hp, start=True,
                         stop=True, perf_mode=DR, skip_group_check=True)
        nc.tensor.matmul(pdt[32:35, :], sb["w8f"], F1, start=True,
                         stop=True, skip_group_check=True)
        stg = work.tile([35, B], F32, name="stg", tag="stg")
        nc.vector.tensor_copy(stg, pdt[0:35, :])
        nc.sync.dma_start(out=DTH[s : s + 1, 0, :], in_=stg[0:1, :])
        nc.sync.dma_start(out=DTH[s : s + 1, 1:4, :], in_=stg[32:35, :])

    # ---- chunk loop, group-pipelined ----
    encg = emit_angle_group(0)
    for g in range(N_GROUPS):
        cur = encg
        if g + 1 < N_GROUPS:
            encg = emit_angle_group(g + 1)
        for j in range(4):
            for l in range(2):
                s = 16 * j + 2 * g + l
                chunk_mlp(s, cur[:, 2 * j + l, :])

    # ---- phase 2: compositing ----
    SG = pre.tile([S, B], F32, name="SG", tag="SG")
    nc.scalar.activation(out=SG, in_=DTH[:, 0, :], func=AF.Relu,
                         bias=sb["b7d64"])
    M64 = pre.tile([S, B], BF16, name="M64", tag="M64")
    nc.gpsimd.tensor_mul(M64, SG, dists)

    mcum = psA.tile([128, 2, B], F32, name="mmA", tag="mmA")[:S, 0, :]
    nc.tensor.matmul(mcum, sb["ltri"], M64, start=True, stop=True)
    vcum = psA.tile([128, 2, B], F32, name="mmA", tag="mmA")[:S, 0, :]
    nc.tensor.matmul(vcum, sb["ltri2"], M64, start=True, stop=True)
    T64 = pre.tile([S, B], F32, name="T64", tag="T64")
    nc.scalar.activation(out=T64, in_=mcum, func=AF.Exp, scale=-1.0)
    T64b = pre.tile([S, B], F32, name="T64b", tag="T64b")
    nc.scalar.activation(out=T64b, in_=vcum, func=AF.Exp, scale=-1.0)
    w64 = pre.tile([S, B], F32, name="w64", tag="w64")
    nc.vector.tensor_sub(w64, T64, T64b)

    # out_c = sum_s 0.5 * (w_s + w_s * tanh((TH_cs + cv_c)/2))
    for c in range(3):
        P8 = pre.tile([S, B], F32, name=f"P8{c}", tag=f"P8{c}")
        nc.gpsimd.tensor_add(P8, DTH[:, 1 + c, :], CVB[c])
        THt = pre.tile([S, B], F32, name=f"THt{c}", tag=f"THt{c}")
        nc.scalar.activation(out=THt, in_=P8, func=AF.Tanh, scale=0.5)
        P = pre.tile([S, B], BF16, name=f"P{c}", tag=f"P{c}")
        nc.vector.tensor_mul(P, w64, THt)
        nc.vector.tensor_add(P, P, w64)
        pc = psA.tile([128, 2, B], F32, name="mmA", tag="mmA")[0:1, 0, :]
        nc.tensor.matmul(pc, sb["half641"], P, start=True, stop=True)
        oc = pre.tile([1, B], F32, name=f"oc{c}", tag=f"oc{c}")
        nc.scalar.activation(out=oc, in_=pc, func=AF.Copy)
        nc.sync.dma_start(out=out_ap.rearrange("b c -> c b")[c : c + 1], in_=oc)


def build_nc():
    nc = bacc.Bacc("TRN2", target_bir_lowering=False, debug=False)
    specs = input_specs()
    aps = {
        name: nc.dram_tensor(name, list(shape), dt, kind="ExternalInput").ap()
        for name, (shape, dt) in specs.items()
    }
    out = nc.dram_tensor("out", [BL, 3], F32, kind="ExternalOutput").ap()
    with tile.TileContext(nc) as tc, ExitStack() as ctx:
        build_nerf(tc, ctx, out, aps)
    nc.compile()
    return nc


def make_in_maps(inputs):
    consts = host_constants()
    wts = host_weights(inputs)
    shared = {**consts, **wts}
    specs = input_specs()
    np_dt = {F32: np.float32, BF16: NP_BF16, F8: NP_F8}
    shared = {
        k: np.ascontiguousarray(np.asarray(v).astype(np_dt[specs[k][1]]))
        for k, v in shared.items()
    }
    in_maps = []
    for core in range(N_CORES):
        sl = slice(core * BL, (core + 1) * BL)
        m = dict(shared)
        m["xT"] = np.ascontiguousarray(np.asarray(inputs["x"])[sl].T, dtype=np.float32)
        m["off"] = np.ascontiguousarray(
            np.asarray(inputs["offsets"])[:, sl], dtype=np.float32
        )
        in_maps.append(m)
    return in_maps


def kernel(**inputs):
    from concourse.bass_utils import run_bass_kernel_spmd

    nc = build_nc()
    in_maps = make_in_maps(inputs)
    res = run_bass_kernel_spmd(nc, in_maps, core_ids=list(range(N_CORES)))
    out = np.concatenate([r["out"] for r in res.results], axis=0)
    return out.astype(np.float32)


# revision 9
# speedup vs baseline: 1.6172x; 1.6172x over previous
"""NeRF MLP kernel for Trainium2 (Bass/Tile), 8-core data-parallel over rays.

v2: fp8e4 DoubleRow matmuls for every K=256 contraction (one PE pass
instead of two), bf16 for the K=30 encoder taps and the color head, and a
batched positional-encoding path.

Device layout: features on SBUF partitions, rays on the free dim.  One
chunk = one sample index s for all 512 local rays.

Key choices vs the f32r baseline:
- Weights are quantized host-side: e4m3 for the 256-wide hidden layers
  (fed to the PE in DoubleRow mode: rhs [128,2,B] packs the two K-halves,
  halving matmuls per layer), bf16 elsewhere.  Density margin to the
  relu cut is ~0.03 while the quantization shift is <4e-3, so the
  all-transparent case still yields an exactly-zero output.
- The angle path runs on groups of 8 chunks strip-packed into 128
  partitions ([128,{2,B}] tiles, 4 strips of 30 rows), so the DVE/ACT
  cost per chunk drops ~4x; a small DMA repacks sin() output to the
  [30, 8*B] layout the encoder matmuls consume.
- GpSimd (no PSUM port) takes the SBUF-only elementwise work; PSUM
  evacuations alternate ScalarE/VectorE by a static table.
- view-dir color contribution (w8v) is per-ray constant: computed once
  before the chunk loop and folded into the phase-2 tanh input, removing
  one matmul per chunk.
- Compositing identical to baseline: w_s = exp(-M_s) - exp(-(M_s+m_s))
  via two triangular matmuls (bf16 lhsT; exp(0)=1 exactly so transparent
  rays give exactly zero).
"""

import math
from contextlib import ExitStack

import numpy as np
import ml_dtypes

import concourse.bass as bass
import concourse.mybir as mybir
import concourse.tile as tile
from concourse import bacc

F32 = mybir.dt.float32
BF16 = mybir.dt.bfloat16
F8 = mybir.dt.float8e4
AF = mybir.ActivationFunctionType
OP = mybir.AluOpType
DR = mybir.MatmulPerfMode.DoubleRow

NP_BF16 = ml_dtypes.bfloat16
NP_F8 = ml_dtypes.float8_e4m3

S = 64          # samples per ray
B_FULL = 4096   # total rays
N_CORES = 8
BL = B_FULL // N_CORES  # rays per core = 512
H = 256
NEAR, FAR = 2.0, 6.0
DELTA = (FAR - NEAR) / S
L_ENC = 5
ENC = 3 * L_ENC * 2  # 30
PI = math.pi
TWO_PI = 2.0 * math.pi
MAGIC = 12582912.0  # 1.5 * 2**23, fp32 round-to-nearest trick

N_GROUPS = 8  # angle groups: 4 partition strips x 2 slots = 8 chunks each

# psum evacuation engine per (layer, half): 7 ACT + 7 DVE halves,
# F1 on ACT and the stage copy on DVE make it 8/8 per chunk.
EVAC_ENG = {i: ("act", "dve") for i in range(7)}


def host_constants():
    c = {}
    freqs = (2.0 ** (np.arange(L_ENC, dtype=np.float64) - 2)) * math.pi
    fturn = np.zeros((ENC, 1), dtype=np.float32)
    phase = np.zeros((ENC, 1), dtype=np.float32)
    for cc in range(3):
        for ll in range(L_ENC):
            for tt in range(2):
                j = cc * (L_ENC * 2) + ll * 2 + tt
                fturn[j, 0] = freqs[ll] / TWO_PI
                phase[j, 0] = 0.0 if tt == 0 else 0.25
    c["fturn30"] = fturn
    c["phase30"] = phase
    c["cap1e10"] = np.full((1, BL), 1.0e10, dtype=np.float32)
    c["svec64"] = (NEAR + np.arange(S, dtype=np.float32)[:, None] * DELTA).astype(
        np.float32
    )
    c["ltri"] = np.triu(np.ones((S, S))).astype(NP_BF16)
    c["ltri2"] = (np.triu(np.ones((S, S))) + np.eye(S)).astype(NP_BF16)
    c["ones31"] = np.ones((3, 1), dtype=np.float32)
    c["half641"] = np.full((S, 1), 0.5, dtype=NP_BF16)
    return c


def host_weights(inp):
    w = {}

    def kstack(m):  # [256, M] -> [128, 2, M]
        return np.ascontiguousarray(m.reshape(2, 128, m.shape[1]).transpose(1, 0, 2))

    w["w0s"] = np.asarray(inp["w0"]).astype(NP_BF16)             # [30, 256]
    for i in (1, 2, 3, 5, 6):
        w[f"w{i}s"] = kstack(np.asarray(inp[f"w{i}"])).astype(NP_F8)
    w["w4h"] = kstack(np.asarray(inp["w4"])[0:H]).astype(NP_F8)  # [128, 2, 256]
    w["w4e"] = np.asarray(inp["w4"])[H : H + ENC].astype(NP_BF16)
    w["w7f"] = kstack(np.asarray(inp["w7"])[:, 1:129]).astype(NP_F8)
    w7d = np.zeros((128, 2, 16), dtype=np.float32)
    w7d[:, :, 0:1] = kstack(np.asarray(inp["w7"])[:, 0:1])
    w["w7d"] = w7d.astype(NP_F8)
    w["w8f"] = np.asarray(inp["w8"])[0:128].astype(NP_BF16)      # [128, 3]
    w["w8v"] = np.asarray(inp["w8"])[128:131].astype(NP_BF16)    # [3, 3]
    for i in range(7):
        w[f"b{i}s"] = np.ascontiguousarray(
            np.asarray(inp[f"b{i}"]).reshape(2, 128).T
        ).astype(np.float32)                                     # [128, 2]
    w["b7f"] = np.ascontiguousarray(
        np.asarray(inp["b7"])[1:129, None]).astype(np.float32)
    w["b7d64"] = np.full((S, 1), np.asarray(inp["b7"])[0], dtype=np.float32)
    w["b8vec3"] = np.ascontiguousarray(
        np.asarray(inp["b8"])[:, None]).astype(np.float32)
    return w


def input_specs():
    """name -> (shape, dtype) for every ExternalInput."""
    specs = {
        "xT": ((6, BL), F32),
        "off": ((S, BL), F32),
        "w0s": ((30, 256), BF16),
        "w4h": ((128, 2, 256), F8),
        "w4e": ((30, 256), BF16),
        "w7f": ((128, 2, 128), F8),
        "w7d": ((128, 2, 16), F8),
        "w8f": ((128, 3), BF16),
        "w8v": ((3, 3), BF16),
        "b7f": ((128, 1), F32),
        "b7d64": ((S, 1), F32),
        "b8vec3": ((3, 1), F32),
        "cap1e10": ((1, BL), F32),
        "fturn30": ((ENC, 1), F32),
        "phase30": ((ENC, 1), F32),
        "svec64": ((S, 1), F32),
        "ltri": ((S, S), BF16),
        "ltri2": ((S, S), BF16),
        "ones31": ((3, 1), F32),
        "half641": ((S, 1), BF16),
    }
    for i in (1, 2, 3, 5, 6):
        specs[f"w{i}s"] = ((128, 2, 256), F8)
    for i in range(7):
        specs[f"b{i}s"] = ((128, 2), F32)
    return specs


CONST_NAMES = tuple(sorted(input_specs().keys() - {"xT", "off"}))


def bcast_rows(ap, reps, cols):
    """Source AP repeating each row of `ap` `reps` times (for DMA)."""
    rows = ap.shape[0]
    return bass.AP(
        tensor=ap.tensor,
        offset=ap.offset,
        ap=[[ap.ap[0][0], rows], [0, reps], [1, cols]],
    )


def rep_free(ap, reps, cols):
    """[P, cols] -> [P, reps, cols] with 0-stride free-dim repetition."""
    return bass.AP(
        tensor=ap.tensor,
        offset=ap.offset,
        ap=[list(ap.ap[0]), [0, reps], [1, cols]],
    )


def build_nerf(tc, ctx, out_ap, a):
    nc = tc.nc
    B = BL

    consts = ctx.enter_context(tc.tile_pool(name="consts", bufs=1))
    pre = ctx.enter_context(tc.tile_pool(name="pre", bufs=1))
    agrp = ctx.enter_context(tc.tile_pool(name="agrp", bufs=2))
    work = ctx.enter_context(tc.tile_pool(name="work", bufs=3))
    psA = ctx.enter_context(tc.tile_pool(name="psA", bufs=4, space="PSUM"))

    # ---- constants / weights ----
    sb = {}
    specs = input_specs()
    for name in CONST_NAMES:
        t = consts.tile(list(specs[name][0]), specs[name][1], name=name, tag=name)
        nc.sync.dma_start(out=t, in_=a[name])
        sb[name] = t

    dt3 = pre.tile([3, B], F32, name="dt3", tag="dt3")
    nc.sync.dma_start(out=dt3, in_=a["xT"][3:6])
    off = pre.tile([S, B], F32, name="off", tag="off")
    nc.sync.dma_start(out=off, in_=a["off"])

    # per-ray encoding constants: angle/2pi = DF*z + AO
    D30 = pre.tile([ENC, B], F32, name="D30", tag="D30")
    nc.sync.dma_start(out=D30, in_=bcast_rows(a["xT"][3:6], 2 * L_ENC, B))
    O30 = pre.tile([ENC, B], F32, name="O30", tag="O30")
    nc.sync.dma_start(out=O30, in_=bcast_rows(a["xT"][0:3], 2 * L_ENC, B))
    DF = pre.tile([ENC, B], F32, name="DF", tag="DF")
    nc.vector.tensor_scalar(out=DF, in0=D30, scalar1=sb["fturn30"],
                            scalar2=None, op0=OP.mult)
    AO = pre.tile([ENC, B], F32, name="AO", tag="AO")
    nc.vector.tensor_scalar(out=AO, in0=O30, scalar1=sb["fturn30"],
                            scalar2=sb["phase30"], op0=OP.mult, op1=OP.add)

    # strip-replicated DF/AO for the batched angle path
    DF2 = pre.tile([128, B], F32, name="DF2", tag="DF2")
    AO2 = pre.tile([128, B], F32, name="AO2", tag="AO2")
    for j in range(4):
        nc.sync.dma_start(out=DF2[32 * j : 32 * j + ENC], in_=DF)
        nc.sync.dma_start(out=AO2[32 * j : 32 * j + ENC], in_=AO)

    # Z[s, b] = NEAR + (s + off) * DELTA
    Z = pre.tile([S, B], F32, name="Z", tag="Z")
    nc.vector.tensor_scalar(out=Z, in0=off, scalar1=DELTA, scalar2=sb["svec64"],
                            op0=OP.mult, op1=OP.add)

    # |d| and view_dir
    sq3 = pre.tile([3, B], F32, name="sq3", tag="sq3")
    nc.vector.tensor_mul(sq3, dt3, dt3)
    ps_nd = psA.tile([128, 2, B], F32, name="mmA", tag="mmA")[0:1, 0, :]
    nc.tensor.matmul(ps_nd, sb["ones31"], sq3, start=True, stop=True)
    nd = pre.tile([1, B], F32, name="nd", tag="nd")
    nc.scalar.activation(out=nd, in_=ps_nd, func=AF.Sqrt)
    inv_nd = pre.tile([1, B], F32, name="inv_nd", tag="inv_nd")
    nc.vector.reciprocal(out=inv_nd, in_=nd)
    inv3 = pre.tile([3, B], F32, name="inv3", tag="inv3")
    nc.gpsimd.partition_broadcast(inv3, inv_nd)
    v3 = pre.tile([3, B], BF16, name="v3", tag="v3")
    nc.vector.tensor_mul(v3, dt3, inv3)

    # per-ray color offset cv_c = (w8v.T v3)_c + b8_c, broadcast to 64 rows
    ps_cv = psA.tile([128, 2, B], F32, name="mmA", tag="mmA")[0:3, 0, :]
    nc.tensor.matmul(ps_cv, sb["w8v"], v3, start=True, stop=True)
    CVS = pre.tile([3, B], F32, name="CVS", tag="CVS")
    nc.vector.tensor_scalar(out=CVS, in0=ps_cv, scalar1=sb["b8vec3"],
                            scalar2=None, op0=OP.add)
    CVB = []
    for c in range(3):
        t = pre.tile([S, B], F32, name=f"CVB{c}", tag=f"CVB{c}")
        nc.sync.dma_start(out=t, in_=bcast_rows(CVS[c : c + 1], S, B))
        CVB.append(t)

    # dists
    nd64 = pre.tile([S, B], F32, name="nd64", tag="nd64")
    nc.gpsimd.partition_broadcast(nd64, nd)
    ZN = pre.tile([S, B], F32, name="ZN", tag="ZN")
    nc.vector.tensor_mul(ZN, Z, nd64)
    ZNs = pre.tile([S, B], F32, name="ZNs", tag="ZNs")
    nc.sync.dma_start(out=ZNs[0 : S - 1], in_=ZN[1:S])
    nc.sync.dma_start(out=ZNs[S - 1 : S], in_=a["cap1e10"])
    dists = pre.tile([S, B], F32, name="dists", tag="dists")
    nc.vector.tensor_sub(dists, ZNs, ZN)

    # raw density + color-tanh rows, written during the chunk loop
    DTH = pre.tile([S, 4, B], F32, name="DTH", tag="DTH")

    def emit_angle_group(g):
        """Angle path for chunks {16j + 2g + l}: strip-packed [128,{2,B}]."""
        zg = agrp.tile([128, 2, B], F32, name="zg", tag="zg")
        for j in range(4):
            for l in range(2):
                s = 16 * j + 2 * g + l
                nc.sync.dma_start(
                    out=zg[32 * j : 32 * j + ENC, l, :],
                    in_=bcast_rows(Z[s : s + 1], ENC, B),
                )
        u = agrp.tile([128, 2, B], F32, name="u", tag="u")
        nc.gpsimd.tensor_mul(u, zg, rep_free(DF2, 2, B))
        nc.gpsimd.tensor_add(u, u, rep_free(AO2, 2, B))
        kk = agrp.tile([128, 2, B], F32, name="kk", tag="kk")
        nc.vector.tensor_scalar(out=kk, in0=u, scalar1=MAGIC, scalar2=MAGIC,
                                op0=OP.add, op1=OP.subtract)
        nc.vector.tensor_sub(u, u, kk)
        encs = agrp.tile([128, 2, B], BF16, name="encs", tag="encs")
        nc.scalar.activation(out=encs, in_=u, func=AF.Sin, scale=TWO_PI)
        encg = agrp.tile([ENC, 8, B], BF16, name="encg", tag="encg")
        for j in range(4):
            nc.sync.dma_start(
                out=encg[:, 2 * j : 2 * j + 2, :],
                in_=encs[32 * j : 32 * j + ENC, :, :],
            )
        return encg

    def evac(p, h, bname, engines):
        for m, eng in enumerate(engines):
            bias = sb[bname][:, m : m + 1]
            if eng == "act":
                nc.scalar.activation(out=h[:, m, :], in_=p[:, m, :],
                                     func=AF.Relu, bias=bias)
            else:
                nc.vector.tensor_scalar(
                    out=h[:, m, :], in0=p[:, m, :], scalar1=bias, scalar2=0.0,
                    op0=OP.add, op1=OP.max,
                )

    def pair_mlp(chunks):
        """Layer-interleaved MLP for a pair of chunks [(s, enc_s, cid), ...].

        Emitting each layer's matmuls for both chunks back-to-back lets one
        chunk's PE work hide the other chunk's PSUM-evacuation latency."""
        st = [{"enc": e, "s": s, "cid": cid} for (s, e, cid) in chunks]

        def mm_layer(li, c):
            p = psA.tile([128, 2, B], F32, name="mmA", tag="mmA")
            if li == 0:
                for m in (0, 1):
                    nc.tensor.matmul(p[:, m, :],
                                     sb["w0s"][:, m * 128 : (m + 1) * 128],
                                     c["enc"], start=True, stop=True)
            elif li == 4:
                for m in (0, 1):
                    nc.tensor.matmul(p[:, m, :],
                                     sb["w4h"][:, :, m * 128 : (m + 1) * 128],
                                     c["h"], start=True, stop=False,
                                     perf_mode=DR, skip_group_check=True)
                    nc.tensor.matmul(p[:, m, :],
                                     sb["w4e"][:, m * 128 : (m + 1) * 128],
                                     c["enc"], start=False, stop=True,
                                     skip_group_check=True)
            else:
                w = sb[f"w{li}s"]
                for m in (0, 1):
                    nc.tensor.matmul(p[:, m, :],
                                     w[:, :, m * 128 : (m + 1) * 128],
                                     c["h"], start=True, stop=True,
                                     perf_mode=DR)
            c["p"] = p

        for li in range(7):
            for c in st:
                mm_layer(li, c)
            for c in st:
                h = work.tile([128, 2, B], F8, name=f"h{li}",
                              tag=f"h{c['cid']}")
                evac(c["p"], h, f"b{li}s", EVAC_ENG[li])
                c["h"] = h

        # L7/L8 tail: features (bank 0), density+color rows (bank 1)
        for c in st:
            p7 = psA.tile([128, 2, B], F32, name="mmA", tag="mmA")
            nc.tensor.matmul(p7[:, 0, :], sb["w7f"], c["h"], start=True,
                             stop=True, perf_mode=DR)
            nc.tensor.matmul(p7[0:16, 1, :], sb["w7d"], c["h"], start=True,
                             stop=True, perf_mode=DR, skip_group_check=True)
            c["p7"] = p7
        for c in st:
            F1 = work.tile([128, B], BF16, name="F1", tag=f"F1{c['cid']}")
            nc.scalar.activation(out=F1, in_=c["p7"][:, 0, :], func=AF.Relu,
                                 bias=sb["b7f"])
            c["F1"] = F1
        for c in st:
            nc.tensor.matmul(c["p7"][32:35, 1, :], sb["w8f"], c["F1"],
                             start=True, stop=True, skip_group_check=True)
        for c in st:
            stg = work.tile([35, B], F32, name="stg", tag=f"stg{c['cid']}")
            nc.vector.tensor_copy(stg, c["p7"][0:35, 1, :])
            s = c["s"]
            nc.sync.dma_start(out=DTH[s : s + 1, 0, :], in_=stg[0:1, :])
            nc.sync.dma_start(out=DTH[s : s + 1, 1:4, :], in_=stg[32:35, :])

    # ---- chunk loop, group-pipelined, pair-interleaved ----
    encg = emit_angle_group(0)
    for g in range(N_GROUPS):
        cur = encg
        if g + 1 < N_GROUPS:
            encg = emit_angle_group(g + 1)
        for j in range(4):
            pair = []
            for l in range(2):
                s = 16 * j + 2 * g + l
                pair.append((s, cur[:, 2 * j + l, :], l))
            pair_mlp(pair)

    # ---- phase 2: compositing ----
    SG = pre.tile([S, B], F32, name="SG", tag="SG")
    nc.scalar.activation(out=SG, in_=DTH[:, 0, :], func=AF.Relu,
                         bias=sb["b7d64"])
    M64 = pre.tile([S, B], BF16, name="M64", tag="M64")
    nc.gpsimd.tensor_mul(M64, SG, dists)

    mcum = psA.tile([128, 2, B], F32, name="mmA", tag="mmA")[:S, 0, :]
    nc.tensor.matmul(mcum, sb["ltri"], M64, start=True, stop=True)
    vcum = psA.tile([128, 2, B], F32, name="mmA", tag="mmA")[:S, 0, :]
    nc.tensor.matmul(vcum, sb["ltri2"], M64, start=True, stop=True)
    T64 = pre.tile([S, B], F32, name="T64", tag="T64")
    nc.scalar.activation(out=T64, in_=mcum, func=AF.Exp, scale=-1.0)
    T64b = pre.tile([S, B], F32, name="T64b", tag="T64b")
    nc.scalar.activation(out=T64b, in_=vcum, func=AF.Exp, scale=-1.0)
    w64 = pre.tile([S, B], F32, name="w64", tag="w64")
    nc.vector.tensor_sub(w64, T64, T64b)

    # out_c = sum_s 0.5 * (w_s + w_s * tanh((TH_cs + cv_c)/2))
    for c in range(3):
        P8 = pre.tile([S, B], F32, name=f"P8{c}", tag=f"P8{c}")
        nc.gpsimd.tensor_add(P8, DTH[:, 1 + c, :], CVB[c])
        THt = pre.tile([S, B], F32, name=f"THt{c}", tag=f"THt{c}")
        nc.scalar.activation(out=THt, in_=P8, func=AF.Tanh, scale=0.5)
        P = pre.tile([S, B], BF16, name=f"P{c}", tag=f"P{c}")
        nc.vector.tensor_mul(P, w64, THt)
        nc.vector.tensor_add(P, P, w64)
        pc = psA.tile([128, 2, B], F32, name="mmA", tag="mmA")[0:1, 0, :]
        nc.tensor.matmul(pc, sb["half641"], P, start=True, stop=True)
        oc = pre.tile([1, B], F32, name=f"oc{c}", tag=f"oc{c}")
        nc.scalar.activation(out=oc, in_=pc, func=AF.Copy)
        nc.sync.dma_start(out=out_ap.rearrange("b c -> c b")[c : c + 1], in_=oc)


def build_nc():
    nc = bacc.Bacc("TRN2", target_bir_lowering=False, debug=False)
    specs = input_specs()
    aps = {
        name: nc.dram_tensor(name, list(shape), dt, kind="ExternalInput").ap()
        for name, (shape, dt) in specs.items()
    }
    out = nc.dram_tensor("out", [BL, 3], F32, kind="ExternalOutput").ap()
    with tile.TileContext(nc) as tc, ExitStack() as ctx:
        build_nerf(tc, ctx, out, aps)
    nc.compile()
    return nc


def make_in_maps(inputs):
    consts = host_constants()
    wts = host_weights(inputs)
    shared = {**consts, **wts}
    specs = input_specs()
    np_dt = {F32: np.float32, BF16: NP_BF16, F8: NP_F8}
    shared = {
        k: np.ascontiguousarray(np.asarray(v).astype(np_dt[specs[k][1]]))
        for k, v in shared.items()
    }
    in_maps = []
    for core in range(N_CORES):
        sl = slice(core * BL, (core + 1) * BL)
        m = dict(shared)
        m["xT"] = np.ascontiguousarray(np.asarray(inputs["x"])[sl].T, dtype=np.float32)
        m["off"] = np.ascontiguousarray(
            np.asarray(inputs["offsets"])[:, sl], dtype=np.float32
        )
        in_maps.append(m)
    return in_maps


def kernel(**inputs):
    from concourse.bass_utils import run_bass_kernel_spmd

    nc = build_nc()
    in_maps = make_in_maps(inputs)
    res = run_bass_kernel_spmd(nc, in_maps, core_ids=list(range(N_CORES)))
    out = np.concatenate([r["out"] for r in res.results], axis=0)
    return out.astype(np.float32)


# revision 13
# speedup vs baseline: 2.2366x; 1.3830x over previous
"""NeRF MLP kernel for Trainium2 (Bass/Tile), 8-core data-parallel over rays.

v2: fp8e4 DoubleRow matmuls for every K=256 contraction (one PE pass
instead of two), bf16 for the K=30 encoder taps and the color head, and a
batched positional-encoding path.

Device layout: features on SBUF partitions, rays on the free dim.  One
chunk = one sample index s for all 512 local rays.

Key choices vs the f32r baseline:
- Weights are quantized host-side: e4m3 for the 256-wide hidden layers
  (fed to the PE in DoubleRow mode: rhs [128,2,B] packs the two K-halves,
  halving matmuls per layer), bf16 elsewhere.  Density margin to the
  relu cut is ~0.03 while the quantization shift is <4e-3, so the
  all-transparent case still yields an exactly-zero output.
- The angle path runs on groups of 8 chunks strip-packed into 128
  partitions ([128,{2,B}] tiles, 4 strips of 30 rows), so the DVE/ACT
  cost per chunk drops ~4x; a small DMA repacks sin() output to the
  [30, 8*B] layout the encoder matmuls consume.
- GpSimd (no PSUM port) takes the SBUF-only elementwise work; PSUM
  evacuations alternate ScalarE/VectorE by a static table.
- view-dir color contribution (w8v) is per-ray constant: computed once
  before the chunk loop and folded into the phase-2 tanh input, removing
  one matmul per chunk.
- Compositing identical to baseline: w_s = exp(-M_s) - exp(-(M_s+m_s))
  via two triangular matmuls (bf16 lhsT; exp(0)=1 exactly so transparent
  rays give exactly zero).
"""

import math
from contextlib import ExitStack

import numpy as np
import ml_dtypes

import concourse.bass as bass
import concourse.mybir as mybir
import concourse.tile as tile
from concourse import bacc

F32 = mybir.dt.float32
BF16 = mybir.dt.bfloat16
F8 = mybir.dt.float8e4
AF = mybir.ActivationFunctionType
OP = mybir.AluOpType
DR = mybir.MatmulPerfMode.DoubleRow

NP_BF16 = ml_dtypes.bfloat16
NP_F8 = ml_dtypes.float8_e4m3

S = 64          # samples per ray
B_FULL = 4096   # total rays
N_CORES = 8
BL = B_FULL // N_CORES  # rays per core = 512
H = 256
NEAR, FAR = 2.0, 6.0
DELTA = (FAR - NEAR) / S
L_ENC = 5
ENC = 3 * L_ENC * 2  # 30
PI = math.pi
TWO_PI = 2.0 * math.pi
MAGIC = 12582912.0  # 1.5 * 2**23, fp32 round-to-nearest trick

N_GROUPS = 8  # angle groups: 4 partition strips x 2 slots = 8 chunks each

# psum evacuation engine per (layer, half): 7 ACT + 7 DVE halves,
# F1 on ACT and the stage copy on DVE make it 8/8 per chunk.
EVAC_ENG = {i: ("act", "dve") for i in range(7)}


def host_constants():
    c = {}
    freqs = (2.0 ** (np.arange(L_ENC, dtype=np.float64) - 2)) * math.pi
    fturn = np.zeros((ENC, 1), dtype=np.float32)
    phase = np.zeros((ENC, 1), dtype=np.float32)
    for cc in range(3):
        for ll in range(L_ENC):
            for tt in range(2):
                j = cc * (L_ENC * 2) + ll * 2 + tt
                fturn[j, 0] = freqs[ll] / TWO_PI
                phase[j, 0] = 0.0 if tt == 0 else 0.25
    c["fturn30"] = fturn
    c["phase30"] = phase
    c["cap1e10"] = np.full((1, BL), 1.0e10, dtype=np.float32)
    c["svec64"] = (NEAR + np.arange(S, dtype=np.float32)[:, None] * DELTA).astype(
        np.float32
    )
    c["ltri"] = np.triu(np.ones((S, S))).astype(NP_BF16)
    c["ltri2"] = (np.triu(np.ones((S, S))) + np.eye(S)).astype(NP_BF16)
    c["ones31"] = np.ones((3, 1), dtype=np.float32)
    c["half641"] = np.full((S, 1), 0.5, dtype=NP_BF16)
    return c


def host_weights(inp):
    w = {}

    def kstack(m):  # [256, M] -> [128, 2, M]
        return np.ascontiguousarray(m.reshape(2, 128, m.shape[1]).transpose(1, 0, 2))

    def swi(kst):
        """[128, 2, M] -> [128, 2M] DoubleRowSwInterleave layout:
        per row, columns (A[M-1], B[M-1], A[M-2], ..., B[0])."""
        p, _, M = kst.shape
        out = np.empty((p, 2 * M), dtype=kst.dtype)
        rev = kst[:, :, ::-1]
        out[:, 0::2] = rev[:, 0, :]
        out[:, 1::2] = rev[:, 1, :]
        return out

    def swi_halves(kst):
        """[128, 2, 2*Mh] -> [128, 2, 2*Mh]: per output half, interleaved."""
        Mh = kst.shape[2] // 2
        return np.stack([swi(kst[:, :, m * Mh : (m + 1) * Mh])
                         for m in range(2)], axis=1)

    w["w0s"] = np.asarray(inp["w0"]).astype(NP_BF16)             # [30, 256]
    for i in (1, 2, 3, 5, 6):
        w[f"w{i}s"] = kstack(np.asarray(inp[f"w{i}"])).astype(NP_F8)
    w["w4h"] = kstack(np.asarray(inp["w4"])[0:H]).astype(NP_F8)
    w["w4e"] = np.asarray(inp["w4"])[H : H + ENC].astype(NP_BF16)
    w["w7f"] = kstack(np.asarray(inp["w7"])[:, 1:129]).astype(NP_F8)
    w7d = np.zeros((128, 2, 16), dtype=np.float32)
    w7d[:, :, 0:1] = kstack(np.asarray(inp["w7"])[:, 0:1])
    w["w7d"] = w7d.astype(NP_F8)
    w["w8f"] = np.asarray(inp["w8"])[0:128].astype(NP_BF16)      # [128, 3]
    w["w8v"] = np.asarray(inp["w8"])[128:131].astype(NP_BF16)    # [3, 3]
    for i in range(7):
        w[f"b{i}s"] = np.ascontiguousarray(
            np.asarray(inp[f"b{i}"]).reshape(2, 128).T
        ).astype(np.float32)                                     # [128, 2]
    w["b7f"] = np.ascontiguousarray(
        np.asarray(inp["b7"])[1:129, None]).astype(np.float32)
    w["b7d64"] = np.full((S, 1), np.asarray(inp["b7"])[0], dtype=np.float32)
    w["b8vec3"] = np.ascontiguousarray(
        np.asarray(inp["b8"])[:, None]).astype(np.float32)
    return w


def input_specs():
    """name -> (shape, dtype) for every ExternalInput."""
    specs = {
        "xT": ((6, BL), F32),
        "off": ((S, BL), F32),
        "w0s": ((30, 256), BF16),
        "w4h": ((128, 2, 256), F8),
        "w4e": ((30, 256), BF16),
        "w7f": ((128, 2, 128), F8),
        "w7d": ((128, 2, 16), F8),
        "w8f": ((128, 3), BF16),
        "w8v": ((3, 3), BF16),
        "b7f": ((128, 1), F32),
        "b7d64": ((S, 1), F32),
        "b8vec3": ((3, 1), F32),
        "cap1e10": ((1, BL), F32),
        "fturn30": ((ENC, 1), F32),
        "phase30": ((ENC, 1), F32),
        "svec64": ((S, 1), F32),
        "ltri": ((S, S), BF16),
        "ltri2": ((S, S), BF16),
        "ones31": ((3, 1), F32),
        "half641": ((S, 1), BF16),
    }
    for i in (1, 2, 3, 5, 6):
        specs[f"w{i}s"] = ((128, 2, 256), F8)
    for i in range(7):
        specs[f"b{i}s"] = ((128, 2), F32)
    return specs


_PRIORITY = ("fturn30", "phase30", "svec64", "w0s", "b0s", "w1s", "b1s",
             "w2s", "b2s", "w3s", "b3s", "w4h", "w4e", "b4s", "w5s", "b5s",
             "w6s", "b6s", "w7f", "w7d", "b7f", "w8f", "w8v", "b8vec3",
             "cap1e10", "ones31")
CONST_NAMES = _PRIORITY + tuple(sorted(
    input_specs().keys() - {"xT", "off"} - set(_PRIORITY)))


def bcast_rows(ap, reps, cols):
    """Source AP repeating each row of `ap` `reps` times (for DMA)."""
    rows = ap.shape[0]
    return bass.AP(
        tensor=ap.tensor,
        offset=ap.offset,
        ap=[[ap.ap[0][0], rows], [0, reps], [1, cols]],
    )


def rep_free(ap, reps, cols):
    """[P, cols] -> [P, reps, cols] with 0-stride free-dim repetition."""
    return bass.AP(
        tensor=ap.tensor,
        offset=ap.offset,
        ap=[list(ap.ap[0]), [0, reps], [1, cols]],
    )


def build_nerf(tc, ctx, out_ap, a):
    nc = tc.nc
    B = BL

    consts = ctx.enter_context(tc.tile_pool(name="consts", bufs=1))
    pre = ctx.enter_context(tc.tile_pool(name="pre", bufs=1))
    agrp = ctx.enter_context(tc.tile_pool(name="agrp", bufs=2))
    work = ctx.enter_context(tc.tile_pool(name="work", bufs=3))
    psA = ctx.enter_context(tc.tile_pool(name="psA", bufs=4, space="PSUM"))

    # ---- constants / weights ----
    sb = {}
    specs = input_specs()
    for name in CONST_NAMES:
        t = consts.tile(list(specs[name][0]), specs[name][1], name=name, tag=name)
        nc.sync.dma_start(out=t, in_=a[name])
        sb[name] = t

    dt3 = pre.tile([3, B], F32, name="dt3", tag="dt3")
    nc.sync.dma_start(out=dt3, in_=a["xT"][3:6])
    off = pre.tile([S, B], F32, name="off", tag="off")
    nc.sync.dma_start(out=off, in_=a["off"])

    # per-ray encoding constants: angle/2pi = DF*z + AO
    D30 = pre.tile([ENC, B], F32, name="D30", tag="D30")
    nc.sync.dma_start(out=D30, in_=bcast_rows(a["xT"][3:6], 2 * L_ENC, B))
    O30 = pre.tile([ENC, B], F32, name="O30", tag="O30")
    nc.sync.dma_start(out=O30, in_=bcast_rows(a["xT"][0:3], 2 * L_ENC, B))
    DF = pre.tile([ENC, B], F32, name="DF", tag="DF")
    nc.vector.tensor_scalar(out=DF, in0=D30, scalar1=sb["fturn30"],
                            scalar2=None, op0=OP.mult)
    AO = pre.tile([ENC, B], F32, name="AO", tag="AO")
    nc.vector.tensor_scalar(out=AO, in0=O30, scalar1=sb["fturn30"],
                            scalar2=sb["phase30"], op0=OP.mult, op1=OP.add)

    # strip-replicated DF/AO for the batched angle path
    DF2 = pre.tile([128, B], F32, name="DF2", tag="DF2")
    AO2 = pre.tile([128, B], F32, name="AO2", tag="AO2")
    for j in range(4):
        nc.sync.dma_start(out=DF2[32 * j : 32 * j + ENC], in_=DF)
        nc.sync.dma_start(out=AO2[32 * j : 32 * j + ENC], in_=AO)

    # Z[s, b] = NEAR + (s + off) * DELTA
    Z = pre.tile([S, B], F32, name="Z", tag="Z")
    nc.vector.tensor_scalar(out=Z, in0=off, scalar1=DELTA, scalar2=sb["svec64"],
                            op0=OP.mult, op1=OP.add)

    # |d| and view_dir
    sq3 = pre.tile([3, B], F32, name="sq3", tag="sq3")
    nc.vector.tensor_mul(sq3, dt3, dt3)
    ps_nd = psA.tile([128, 2, B], F32, name="mmA", tag="mmA")[0:1, 0, :]
    nc.tensor.matmul(ps_nd, sb["ones31"], sq3, start=True, stop=True)
    nd = pre.tile([1, B], F32, name="nd", tag="nd")
    nc.scalar.activation(out=nd, in_=ps_nd, func=AF.Sqrt)
    inv_nd = pre.tile([1, B], F32, name="inv_nd", tag="inv_nd")
    nc.vector.reciprocal(out=inv_nd, in_=nd)
    inv3 = pre.tile([3, B], F32, name="inv3", tag="inv3")
    nc.gpsimd.partition_broadcast(inv3, inv_nd)
    v3 = pre.tile([3, B], BF16, name="v3", tag="v3")
    nc.vector.tensor_mul(v3, dt3, inv3)

    # per-ray color offset cv_c = (w8v.T v3)_c + b8_c, broadcast to 64 rows
    ps_cv = psA.tile([128, 2, B], F32, name="mmA", tag="mmA")[0:3, 0, :]
    nc.tensor.matmul(ps_cv, sb["w8v"], v3, start=True, stop=True)
    CVS = pre.tile([3, B], F32, name="CVS", tag="CVS")
    nc.vector.tensor_scalar(out=CVS, in0=ps_cv, scalar1=sb["b8vec3"],
                            scalar2=None, op0=OP.add)
    CVB = []
    for c in range(3):
        t = pre.tile([S, B], F32, name=f"CVB{c}", tag=f"CVB{c}")
        nc.sync.dma_start(out=t, in_=bcast_rows(CVS[c : c + 1], S, B))
        CVB.append(t)

    # dists
    nd64 = pre.tile([S, B], F32, name="nd64", tag="nd64")
    nc.gpsimd.partition_broadcast(nd64, nd)
    ZN = pre.tile([S, B], F32, name="ZN", tag="ZN")
    nc.vector.tensor_mul(ZN, Z, nd64)
    ZNs = pre.tile([S, B], F32, name="ZNs", tag="ZNs")
    nc.sync.dma_start(out=ZNs[0 : S - 1], in_=ZN[1:S])
    nc.sync.dma_start(out=ZNs[S - 1 : S], in_=a["cap1e10"])
    dists = pre.tile([S, B], F32, name="dists", tag="dists")
    nc.vector.tensor_sub(dists, ZNs, ZN)

    # raw density + color-tanh rows, written during the chunk loop
    DTH = pre.tile([S, 4, B], F32, name="DTH", tag="DTH")

    def emit_angle_group(g):
        """Angle path for chunks {16j + 2g + l}: strip-packed [128,{2,B}]."""
        zg = agrp.tile([128, 2, B], F32, name="zg", tag="zg")
        for j in range(4):
            for l in range(2):
                s = 16 * j + 2 * g + l
                nc.sync.dma_start(
                    out=zg[32 * j : 32 * j + ENC, l, :],
                    in_=bcast_rows(Z[s : s + 1], ENC, B),
                )
        u = agrp.tile([128, 2, B], F32, name="u", tag="u")
        nc.gpsimd.tensor_mul(u, zg, rep_free(DF2, 2, B))
        nc.gpsimd.tensor_add(u, u, rep_free(AO2, 2, B))
        kk = agrp.tile([128, 2, B], F32, name="kk", tag="kk")
        nc.vector.tensor_scalar(out=kk, in0=u, scalar1=MAGIC, scalar2=MAGIC,
                                op0=OP.add, op1=OP.subtract)
        nc.vector.tensor_sub(u, u, kk)
        encs = agrp.tile([128, 2, B], BF16, name="encs", tag="encs")
        nc.scalar.activation(out=encs, in_=u, func=AF.Sin, scale=TWO_PI)
        encg = agrp.tile([ENC, 8, B], BF16, name="encg", tag="encg")
        for j in range(4):
            nc.sync.dma_start(
                out=encg[:, 2 * j : 2 * j + 2, :],
                in_=encs[32 * j : 32 * j + ENC, :, :],
            )
        return encg

    def evac(p, h, bname, engines):
        for m, eng in enumerate(engines):
            bias = sb[bname][:, m : m + 1]
            if eng == "act":
                nc.scalar.activation(out=h[:, m, :], in_=p[:, m, :],
                                     func=AF.Relu, bias=bias)
            else:
                nc.vector.tensor_scalar(
                    out=h[:, m, :], in0=p[:, m, :], scalar1=bias, scalar2=0.0,
                    op0=OP.add, op1=OP.max,
                )

    def pair_mlp(chunks):
        """Layer-interleaved MLP for a pair of chunks [(s, enc_s, cid), ...].

        Emitting each layer's matmuls for both chunks back-to-back lets one
        chunk's PE work hide the other chunk's PSUM-evacuation latency."""
        st = [{"enc": e, "s": s, "cid": cid} for (s, e, cid) in chunks]

        def mm_layer(li, c):
            p = psA.tile([128, 2, B], F32, name="mmA", tag="mmA")
            if li == 0:
                for m in (0, 1):
                    nc.tensor.matmul(p[:, m, :],
                                     sb["w0s"][:, m * 128 : (m + 1) * 128],
                                     c["enc"], start=True, stop=True)
            elif li == 4:
                for m in (0, 1):
                    nc.tensor.matmul(p[:, m, :],
                                     sb["w4h"][:, :, m * 128 : (m + 1) * 128],
                                     c["h"], start=True, stop=False,
                                     perf_mode=DR, skip_group_check=True)
                    nc.tensor.matmul(p[:, m, :],
                                     sb["w4e"][:, m * 128 : (m + 1) * 128],
                                     c["enc"], start=False, stop=True,
                                     skip_group_check=True)
            else:
                w = sb[f"w{li}s"]
                for m in (0, 1):
                    nc.tensor.matmul(p[:, m, :],
                                     w[:, :, m * 128 : (m + 1) * 128],
                                     c["h"], start=True, stop=True,
                                     perf_mode=DR)
            c["p"] = p

        for li in range(7):
            for c in st:
                mm_layer(li, c)
            for c in st:
                h = work.tile([128, 2, B], F8, name=f"h{li}",
                              tag=f"h{c['cid']}")
                evac(c["p"], h, f"b{li}s", EVAC_ENG[li])
                c["h"] = h

        # L7/L8 tail: features (bank 0), density+color rows (bank 1)
        for c in st:
            p7 = psA.tile([128, 2, B], F32, name="mmA", tag="mmA")
            nc.tensor.matmul(p7[:, 0, :], sb["w7f"], c["h"], start=True,
                             stop=True, perf_mode=DR)
            nc.tensor.matmul(p7[0:16, 1, :], sb["w7d"], c["h"], start=True,
                             stop=True, perf_mode=DR, skip_group_check=True)
            c["p7"] = p7
        for c in st:
            F1 = work.tile([128, B], BF16, name="F1", tag=f"F1{c['cid']}")
            nc.scalar.activation(out=F1, in_=c["p7"][:, 0, :], func=AF.Relu,
                                 bias=sb["b7f"])
            c["F1"] = F1
        for c in st:
            nc.tensor.matmul(c["p7"][32:35, 1, :], sb["w8f"], c["F1"],
                             start=True, stop=True, skip_group_check=True)
        for c in st:
            stg = work.tile([35, B], F32, name="stg", tag=f"stg{c['cid']}")
            nc.vector.tensor_copy(stg, c["p7"][0:35, 1, :])
            s = c["s"]
            nc.sync.dma_start(out=DTH[s : s + 1, 0, :], in_=stg[0:1, :])
            nc.sync.dma_start(out=DTH[s : s + 1, 1:4, :], in_=stg[32:35, :])

    # ---- chunk loop, group-pipelined, pair-interleaved ----
    encg = emit_angle_group(0)
    for g in range(N_GROUPS):
        cur = encg
        if g + 1 < N_GROUPS:
            encg = emit_angle_group(g + 1)
        slots = [(16 * j + 2 * g + l, 2 * j + l) for j in range(4)
                 for l in range(2)]
        for lo in (0, 3, 6):
            grp = [(s, cur[:, slot, :], ci)
                   for ci, (s, slot) in enumerate(slots[lo : lo + 3])]
            pair_mlp(grp)

    # ---- phase 2: compositing ----
    SG = pre.tile([S, B], F32, name="SG", tag="SG")
    nc.scalar.activation(out=SG, in_=DTH[:, 0, :], func=AF.Relu,
                         bias=sb["b7d64"])
    M64 = pre.tile([S, B], BF16, name="M64", tag="M64")
    nc.gpsimd.tensor_mul(M64, SG, dists)

    mcum = psA.tile([128, 2, B], F32, name="mmA", tag="mmA")[:S, 0, :]
    nc.tensor.matmul(mcum, sb["ltri"], M64, start=True, stop=True)
    vcum = psA.tile([128, 2, B], F32, name="mmA", tag="mmA")[:S, 0, :]
    nc.tensor.matmul(vcum, sb["ltri2"], M64, start=True, stop=True)
    T64 = pre.tile([S, B], F32, name="T64", tag="T64")
    nc.scalar.activation(out=T64, in_=mcum, func=AF.Exp, scale=-1.0)
    T64b = pre.tile([S, B], F32, name="T64b", tag="T64b")
    nc.scalar.activation(out=T64b, in_=vcum, func=AF.Exp, scale=-1.0)
    w64 = pre.tile([S, B], F32, name="w64", tag="w64")
    nc.vector.tensor_sub(w64, T64, T64b)

    # out_c = sum_s 0.5 * (w_s + w_s * tanh((TH_cs + cv_c)/2))
    for c in range(3):
        P8 = pre.tile([S, B], F32, name=f"P8{c}", tag=f"P8{c}")
        nc.gpsimd.tensor_add(P8, DTH[:, 1 + c, :], CVB[c])
        THt = pre.tile([S, B], F32, name=f"THt{c}", tag=f"THt{c}")
        nc.scalar.activation(out=THt, in_=P8, func=AF.Tanh, scale=0.5)
        P = pre.tile([S, B], BF16, name=f"P{c}", tag=f"P{c}")
        nc.vector.tensor_mul(P, w64, THt)
        nc.vector.tensor_add(P, P, w64)
        pc = psA.tile([128, 2, B], F32, name="mmA", tag="mmA")[0:1, 0, :]
        nc.tensor.matmul(pc, sb["half641"], P, start=True, stop=True)
        oc = pre.tile([1, B], F32, name=f"oc{c}", tag=f"oc{c}")
        nc.scalar.activation(out=oc, in_=pc, func=AF.Copy)
        nc.sync.dma_start(out=out_ap.rearrange("b c -> c b")[c : c + 1], in_=oc)


def build_nc():
    nc = bacc.Bacc("TRN2", target_bir_lowering=False, debug=False)
    specs = input_specs()
    aps = {
        name: nc.dram_tensor(name, list(shape), dt, kind="ExternalInput").ap()
        for name, (shape, dt) in specs.items()
    }
    out = nc.dram_tensor("out", [BL, 3], F32, kind="ExternalOutput").ap()
    with tile.TileContext(nc) as tc, ExitStack() as ctx:
        build_nerf(tc, ctx, out, aps)
    nc.compile()
    return nc


def make_in_maps(inputs):
    consts = host_constants()
    wts = host_weights(inputs)
    shared = {**consts, **wts}
    specs = input_specs()
    np_dt = {F32: np.float32, BF16: NP_BF16, F8: NP_F8}
    shared = {
        k: np.ascontiguousarray(np.asarray(v).astype(np_dt[specs[k][1]]))
        for k, v in shared.items()
    }
    in_maps = []
    for core in range(N_CORES):
        sl = slice(core * BL, (core + 1) * BL)
        m = dict(shared)
        m["xT"] = np.ascontiguousarray(np.asarray(inputs["x"])[sl].T, dtype=np.float32)
        m["off"] = np.ascontiguousarray(
            np.asarray(inputs["offsets"])[:, sl], dtype=np.float32
        )
        in_maps.append(m)
    return in_maps


def kernel(**inputs):
    from concourse.bass_utils import run_bass_kernel_spmd

    nc = build_nc()
    in_maps = make_in_maps(inputs)
    res = run_bass_kernel_spmd(nc, in_maps, core_ids=list(range(N_CORES)))
    out = np.concatenate([r["out"] for r in res.results], axis=0)
    return out.astype(np.float32)


# revision 15
# speedup vs baseline: 2.8269x; 1.2639x over previous
"""NeRF MLP kernel for Trainium2 (Bass/Tile), 8-core data-parallel over rays.

v2: fp8e4 DoubleRow matmuls for every K=256 contraction (one PE pass
instead of two), bf16 for the K=30 encoder taps and the color head, and a
batched positional-encoding path.

Device layout: features on SBUF partitions, rays on the free dim.  One
chunk = one sample index s for all 512 local rays.

Key choices vs the f32r baseline:
- Weights are quantized host-side: e4m3 for the 256-wide hidden layers
  (fed to the PE in DoubleRow mode: rhs [128,2,B] packs the two K-halves,
  halving matmuls per layer), bf16 elsewhere.  Density margin to the
  relu cut is ~0.03 while the quantization shift is <4e-3, so the
  all-transparent case still yields an exactly-zero output.
- The angle path runs on groups of 8 chunks strip-packed into 128
  partitions ([128,{2,B}] tiles, 4 strips of 30 rows), so the DVE/ACT
  cost per chunk drops ~4x; a small DMA repacks sin() output to the
  [30, 8*B] layout the encoder matmuls consume.
- GpSimd (no PSUM port) takes the SBUF-only elementwise work; PSUM
  evacuations alternate ScalarE/VectorE by a static table.
- view-dir color contribution (w8v) is per-ray constant: computed once
  before the chunk loop and folded into the phase-2 tanh input, removing
  one matmul per chunk.
- Compositing identical to baseline: w_s = exp(-M_s) - exp(-(M_s+m_s))
  via two triangular matmuls (bf16 lhsT; exp(0)=1 exactly so transparent
  rays give exactly zero).
"""

import math
from contextlib import ExitStack

import numpy as np
import ml_dtypes

import concourse.bass as bass
import concourse.mybir as mybir
import concourse.tile as tile
from concourse import bacc

F32 = mybir.dt.float32
BF16 = mybir.dt.bfloat16
F8 = mybir.dt.float8e4
AF = mybir.ActivationFunctionType
OP = mybir.AluOpType
DR = mybir.MatmulPerfMode.DoubleRow

NP_BF16 = ml_dtypes.bfloat16
NP_F8 = ml_dtypes.float8_e4m3

S = 64          # samples per ray
B_FULL = 4096   # total rays
N_CORES = 8
BL = B_FULL // N_CORES  # rays per core = 512
H = 256
NEAR, FAR = 2.0, 6.0
DELTA = (FAR - NEAR) / S
L_ENC = 5
ENC = 3 * L_ENC * 2  # 30
PI = math.pi
TWO_PI = 2.0 * math.pi
MAGIC = 12582912.0  # 1.5 * 2**23, fp32 round-to-nearest trick

N_GROUPS = 8  # angle groups: 4 partition strips x 2 slots = 8 chunks each

# psum evacuation engine per (layer, half): 7 ACT + 7 DVE halves,
# F1 on ACT and the stage copy on DVE make it 8/8 per chunk.
EVAC_ENG = {i: ("act", "dve") for i in range(7)}


def host_constants():
    c = {}
    freqs = (2.0 ** (np.arange(L_ENC, dtype=np.float64) - 2)) * math.pi
    fturn = np.zeros((ENC, 1), dtype=np.float32)
    phase = np.zeros((ENC, 1), dtype=np.float32)
    for cc in range(3):
        for ll in range(L_ENC):
            for tt in range(2):
                j = cc * (L_ENC * 2) + ll * 2 + tt
                fturn[j, 0] = freqs[ll] / TWO_PI
                phase[j, 0] = 0.0 if tt == 0 else 0.25
    c["fturn30"] = fturn
    c["phase30"] = phase
    c["cap1e10"] = np.full((1, BL), 1.0e10, dtype=np.float32)
    c["svec64"] = (NEAR + np.arange(S, dtype=np.float32)[:, None] * DELTA).astype(
        np.float32
    )
    c["ltri"] = np.triu(np.ones((S, S))).astype(NP_BF16)
    c["ltri2"] = (np.triu(np.ones((S, S))) + np.eye(S)).astype(NP_BF16)
    c["ones31"] = np.ones((3, 1), dtype=np.float32)
    c["half641"] = np.full((S, 1), 0.5, dtype=NP_BF16)
    return c


def host_weights(inp):
    w = {}

    def kstack(m):  # [256, M] -> [128, 2, M]
        return np.ascontiguousarray(m.reshape(2, 128, m.shape[1]).transpose(1, 0, 2))

    def swi(kst):
        """[128, 2, M] -> [128, 2M] DoubleRowSwInterleave layout:
        per row, columns (A[M-1], B[M-1], A[M-2], ..., B[0])."""
        p, _, M = kst.shape
        out = np.empty((p, 2 * M), dtype=kst.dtype)
        rev = kst[:, :, ::-1]
        out[:, 0::2] = rev[:, 0, :]
        out[:, 1::2] = rev[:, 1, :]
        return out

    def swi_halves(kst):
        """[128, 2, 2*Mh] -> [128, 2, 2*Mh]: per output half, interleaved."""
        Mh = kst.shape[2] // 2
        return np.stack([swi(kst[:, :, m * Mh : (m + 1) * Mh])
                         for m in range(2)], axis=1)

    w["w0s"] = np.asarray(inp["w0"]).astype(NP_BF16)             # [30, 256]
    for i in (1, 2, 3, 5, 6):
        w[f"w{i}s"] = kstack(np.asarray(inp[f"w{i}"])).astype(NP_F8)
    w["w4h"] = kstack(np.asarray(inp["w4"])[0:H]).astype(NP_F8)
    w["w4e"] = np.asarray(inp["w4"])[H : H + ENC].astype(NP_BF16)
    w["w7f"] = kstack(np.asarray(inp["w7"])[:, 1:129]).astype(NP_F8)
    w7d = np.zeros((128, 2, 16), dtype=np.float32)
    w7d[:, :, 0:1] = kstack(np.asarray(inp["w7"])[:, 0:1])
    w["w7d"] = w7d.astype(NP_F8)
    w["w8f"] = np.asarray(inp["w8"])[0:128].astype(NP_BF16)      # [128, 3]
    w["w8v"] = np.asarray(inp["w8"])[128:131].astype(NP_BF16)    # [3, 3]
    for i in range(7):
        w[f"b{i}s"] = np.ascontiguousarray(
            np.asarray(inp[f"b{i}"]).reshape(2, 128).T
        ).astype(np.float32)                                     # [128, 2]
    w["b7f"] = np.ascontiguousarray(
        np.asarray(inp["b7"])[1:129, None]).astype(np.float32)
    w["b7d64"] = np.full((S, 1), np.asarray(inp["b7"])[0], dtype=np.float32)
    w["b8vec3"] = np.ascontiguousarray(
        np.asarray(inp["b8"])[:, None]).astype(np.float32)
    return w


def input_specs():
    """name -> (shape, dtype) for every ExternalInput."""
    specs = {
        "xT": ((6, BL), F32),
        "off": ((S, BL), F32),
        "w0s": ((30, 256), BF16),
        "w4h": ((128, 2, 256), F8),
        "w4e": ((30, 256), BF16),
        "w7f": ((128, 2, 128), F8),
        "w7d": ((128, 2, 16), F8),
        "w8f": ((128, 3), BF16),
        "w8v": ((3, 3), BF16),
        "b7f": ((128, 1), F32),
        "b7d64": ((S, 1), F32),
        "b8vec3": ((3, 1), F32),
        "cap1e10": ((1, BL), F32),
        "fturn30": ((ENC, 1), F32),
        "phase30": ((ENC, 1), F32),
        "svec64": ((S, 1), F32),
        "ltri": ((S, S), BF16),
        "ltri2": ((S, S), BF16),
        "ones31": ((3, 1), F32),
        "half641": ((S, 1), BF16),
    }
    for i in (1, 2, 3, 5, 6):
        specs[f"w{i}s"] = ((128, 2, 256), F8)
    for i in range(7):
        specs[f"b{i}s"] = ((128, 2), F32)
    return specs


_PRIORITY = ("fturn30", "phase30", "svec64", "w0s", "b0s", "w1s", "b1s",
             "w2s", "b2s", "w3s", "b3s", "w4h", "w4e", "b4s", "w5s", "b5s",
             "w6s", "b6s", "w7f", "w7d", "b7f", "w8f", "w8v", "b8vec3",
             "cap1e10", "ones31")
CONST_NAMES = _PRIORITY + tuple(sorted(
    input_specs().keys() - {"xT", "off"} - set(_PRIORITY)))


def bcast_rows(ap, reps, cols):
    """Source AP repeating each row of `ap` `reps` times (for DMA)."""
    rows = ap.shape[0]
    return bass.AP(
        tensor=ap.tensor,
        offset=ap.offset,
        ap=[[ap.ap[0][0], rows], [0, reps], [1, cols]],
    )


def rep_free(ap, reps, cols):
    """[P, cols] -> [P, reps, cols] with 0-stride free-dim repetition."""
    return bass.AP(
        tensor=ap.tensor,
        offset=ap.offset,
        ap=[list(ap.ap[0]), [0, reps], [1, cols]],
    )


def build_nerf(tc, ctx, out_ap, a):
    nc = tc.nc
    B = BL

    consts = ctx.enter_context(tc.tile_pool(name="consts", bufs=1))
    pre = ctx.enter_context(tc.tile_pool(name="pre", bufs=1))
    agrp = ctx.enter_context(tc.tile_pool(name="agrp", bufs=2))
    work = ctx.enter_context(tc.tile_pool(name="work", bufs=3))
    psA = ctx.enter_context(tc.tile_pool(name="psA", bufs=8, space="PSUM"))

    # ---- constants / weights ----
    sb = {}
    specs = input_specs()
    dma_engines = (nc.sync, nc.gpsimd, nc.scalar)
    for idx, name in enumerate(CONST_NAMES):
        t = consts.tile(list(specs[name][0]), specs[name][1], name=name, tag=name)
        dma_engines[idx % len(dma_engines)].dma_start(out=t, in_=a[name])
        sb[name] = t

    dt3 = pre.tile([3, B], F32, name="dt3", tag="dt3")
    nc.sync.dma_start(out=dt3, in_=a["xT"][3:6])
    off = pre.tile([S, B], F32, name="off", tag="off")
    nc.sync.dma_start(out=off, in_=a["off"])

    # per-ray encoding constants: angle/2pi = DF*z + AO
    D30 = pre.tile([ENC, B], F32, name="D30", tag="D30")
    nc.sync.dma_start(out=D30, in_=bcast_rows(a["xT"][3:6], 2 * L_ENC, B))
    O30 = pre.tile([ENC, B], F32, name="O30", tag="O30")
    nc.sync.dma_start(out=O30, in_=bcast_rows(a["xT"][0:3], 2 * L_ENC, B))
    DF = pre.tile([ENC, B], F32, name="DF", tag="DF")
    nc.vector.tensor_scalar(out=DF, in0=D30, scalar1=sb["fturn30"],
                            scalar2=None, op0=OP.mult)
    AO = pre.tile([ENC, B], F32, name="AO", tag="AO")
    nc.vector.tensor_scalar(out=AO, in0=O30, scalar1=sb["fturn30"],
                            scalar2=sb["phase30"], op0=OP.mult, op1=OP.add)

    # strip-replicated DF/AO for the batched angle path
    DF2 = pre.tile([128, B], F32, name="DF2", tag="DF2")
    AO2 = pre.tile([128, B], F32, name="AO2", tag="AO2")
    for j in range(4):
        nc.sync.dma_start(out=DF2[32 * j : 32 * j + ENC], in_=DF)
        nc.sync.dma_start(out=AO2[32 * j : 32 * j + ENC], in_=AO)

    # Z[s, b] = NEAR + (s + off) * DELTA
    Z = pre.tile([S, B], F32, name="Z", tag="Z")
    nc.vector.tensor_scalar(out=Z, in0=off, scalar1=DELTA, scalar2=sb["svec64"],
                            op0=OP.mult, op1=OP.add)

    # |d| and view_dir
    sq3 = pre.tile([3, B], F32, name="sq3", tag="sq3")
    nc.vector.tensor_mul(sq3, dt3, dt3)
    ps_nd = psA.tile([128, B], F32, name="mmA", tag="mmA")[0:1, :]
    nc.tensor.matmul(ps_nd, sb["ones31"], sq3, start=True, stop=True)
    nd = pre.tile([1, B], F32, name="nd", tag="nd")
    nc.scalar.activation(out=nd, in_=ps_nd, func=AF.Sqrt)
    inv_nd = pre.tile([1, B], F32, name="inv_nd", tag="inv_nd")
    nc.vector.reciprocal(out=inv_nd, in_=nd)
    inv3 = pre.tile([3, B], F32, name="inv3", tag="inv3")
    nc.gpsimd.partition_broadcast(inv3, inv_nd)
    v3 = pre.tile([3, B], BF16, name="v3", tag="v3")
    nc.vector.tensor_mul(v3, dt3, inv3)

    # per-ray color offset cv_c = (w8v.T v3)_c + b8_c, broadcast to 64 rows
    ps_cv = psA.tile([128, B], F32, name="mmA", tag="mmA")[0:3, :]
    nc.tensor.matmul(ps_cv, sb["w8v"], v3, start=True, stop=True)
    CVS = pre.tile([3, B], F32, name="CVS", tag="CVS")
    nc.vector.tensor_scalar(out=CVS, in0=ps_cv, scalar1=sb["b8vec3"],
                            scalar2=None, op0=OP.add)
    CVB = []
    for c in range(3):
        t = pre.tile([S, B], F32, name=f"CVB{c}", tag=f"CVB{c}")
        nc.sync.dma_start(out=t, in_=bcast_rows(CVS[c : c + 1], S, B))
        CVB.append(t)

    # dists
    nd64 = pre.tile([S, B], F32, name="nd64", tag="nd64")
    nc.gpsimd.partition_broadcast(nd64, nd)
    ZN = pre.tile([S, B], F32, name="ZN", tag="ZN")
    nc.vector.tensor_mul(ZN, Z, nd64)
    ZNs = pre.tile([S, B], F32, name="ZNs", tag="ZNs")
    nc.sync.dma_start(out=ZNs[0 : S - 1], in_=ZN[1:S])
    nc.sync.dma_start(out=ZNs[S - 1 : S], in_=a["cap1e10"])
    dists = pre.tile([S, B], F32, name="dists", tag="dists")
    nc.vector.tensor_sub(dists, ZNs, ZN)

    # raw density + color-tanh rows, written during the chunk loop
    DTH = pre.tile([S, 4, B], F32, name="DTH", tag="DTH")

    def emit_angle_group(g):
        """Angle path for chunks {16j + 2g + l}: strip-packed [128,{2,B}]."""
        zg = agrp.tile([128, 2, B], F32, name="zg", tag="zg")
        for j in range(4):
            for l in range(2):
                s = 16 * j + 2 * g + l
                nc.gpsimd.dma_start(
                    out=zg[32 * j : 32 * j + ENC, l, :],
                    in_=bcast_rows(Z[s : s + 1], ENC, B),
                )
        u = agrp.tile([128, 2, B], F32, name="u", tag="u")
        nc.gpsimd.tensor_mul(u, zg, rep_free(DF2, 2, B))
        nc.gpsimd.tensor_add(u, u, rep_free(AO2, 2, B))
        kk = agrp.tile([128, 2, B], F32, name="kk", tag="kk")
        nc.vector.tensor_scalar(out=kk, in0=u, scalar1=MAGIC, scalar2=MAGIC,
                                op0=OP.add, op1=OP.subtract)
        nc.vector.tensor_sub(u, u, kk)
        encs = agrp.tile([128, 2, B], BF16, name="encs", tag="encs")
        nc.scalar.activation(out=encs, in_=u, func=AF.Sin, scale=TWO_PI)
        encg = agrp.tile([ENC, 8, B], BF16, name="encg", tag="encg")
        for j in range(4):
            nc.gpsimd.dma_start(
                out=encg[:, 2 * j : 2 * j + 2, :],
                in_=encs[32 * j : 32 * j + ENC, :, :],
            )
        return encg

    def evac(p, h, bname, engines):
        for m, eng in enumerate(engines):
            bias = sb[bname][:, m : m + 1]
            if eng == "act":
                nc.scalar.activation(out=h[:, m, :], in_=p[m],
                                     func=AF.Relu, bias=bias)
            else:
                nc.vector.tensor_scalar(
                    out=h[:, m, :], in0=p[m], scalar1=bias, scalar2=0.0,
                    op0=OP.add, op1=OP.max,
                )

    def pair_mlp(chunks):
        """Layer-interleaved MLP for a pair of chunks [(s, enc_s, cid), ...].

        Emitting each layer's matmuls for both chunks back-to-back lets one
        chunk's PE work hide the other chunk's PSUM-evacuation latency."""
        st = [{"enc": e, "s": s, "cid": cid} for (s, e, cid) in chunks]

        def mm_layer(li, c):
            ph = []
            for m in (0, 1):
                p = psA.tile([128, B], F32, name="mmA", tag="mmA")
                if li == 0:
                    nc.tensor.matmul(p, sb["w0s"][:, m * 128 : (m + 1) * 128],
                                     c["enc"], start=True, stop=True)
                elif li == 4:
                    nc.tensor.matmul(p, sb["w4h"][:, :, m * 128 : (m + 1) * 128],
                                     c["h"], start=True, stop=False,
                                     perf_mode=DR, skip_group_check=True)
                    nc.tensor.matmul(p, sb["w4e"][:, m * 128 : (m + 1) * 128],
                                     c["enc"], start=False, stop=True,
                                     skip_group_check=True)
                else:
                    w = sb[f"w{li}s"]
                    nc.tensor.matmul(p, w[:, :, m * 128 : (m + 1) * 128],
                                     c["h"], start=True, stop=True,
                                     perf_mode=DR)
                ph.append(p)
            c["p"] = ph

        for li in range(7):
            for c in st:
                mm_layer(li, c)
            for c in st:
                h = work.tile([128, 2, B], F8, name=f"h{li}",
                              tag=f"h{c['cid']}")
                evac(c["p"], h, f"b{li}s", EVAC_ENG[li])
                c["h"] = h

        # L7/L8 tail: features (bank 0), density+color rows (bank 1)
        for c in st:
            p7f = psA.tile([128, B], F32, name="mmA", tag="mmA")
            nc.tensor.matmul(p7f, sb["w7f"], c["h"], start=True,
                             stop=True, perf_mode=DR)
            pdt = psA.tile([128, B], F32, name="mmA", tag="mmA")
            nc.tensor.matmul(pdt[0:16, :], sb["w7d"], c["h"], start=True,
                             stop=True, perf_mode=DR, skip_group_check=True)
            c["p7f"], c["pdt"] = p7f, pdt
        for c in st:
            F1 = work.tile([128, B], BF16, name="F1", tag=f"F1{c['cid']}")
            nc.scalar.activation(out=F1, in_=c["p7f"], func=AF.Relu,
                                 bias=sb["b7f"])
            c["F1"] = F1
        for c in st:
            nc.tensor.matmul(c["pdt"][32:35, :], sb["w8f"], c["F1"],
                             start=True, stop=True, skip_group_check=True)
        for c in st:
            stg = work.tile([35, B], F32, name="stg", tag=f"stg{c['cid']}")
            nc.vector.tensor_copy(stg, c["pdt"][0:35, :])
            s = c["s"]
            nc.gpsimd.dma_start(out=DTH[s : s + 1, 0, :], in_=stg[0:1, :])
            nc.gpsimd.dma_start(out=DTH[s : s + 1, 1:4, :], in_=stg[32:35, :])

    # ---- chunk loop, group-pipelined, pair-interleaved ----
    encg = emit_angle_group(0)
    for g in range(N_GROUPS):
        cur = encg
        if g + 1 < N_GROUPS:
            encg = emit_angle_group(g + 1)
        slots = [(16 * j + 2 * g + l, 2 * j + l) for j in range(4)
                 for l in range(2)]
        for lo in (0, 3, 6):
            grp = [(s, cur[:, slot, :], ci)
                   for ci, (s, slot) in enumerate(slots[lo : lo + 3])]
            pair_mlp(grp)

    # ---- phase 2: compositing ----
    SG = pre.tile([S, B], F32, name="SG", tag="SG")
    nc.scalar.activation(out=SG, in_=DTH[:, 0, :], func=AF.Relu,
                         bias=sb["b7d64"])
    M64 = pre.tile([S, B], BF16, name="M64", tag="M64")
    nc.gpsimd.tensor_mul(M64, SG, dists)

    mcum = psA.tile([128, B], F32, name="mmA", tag="mmA")[:S, :]
    nc.tensor.matmul(mcum, sb["ltri"], M64, start=True, stop=True)
    vcum = psA.tile([128, B], F32, name="mmA", tag="mmA")[:S, :]
    nc.tensor.matmul(vcum, sb["ltri2"], M64, start=True, stop=True)
    T64 = pre.tile([S, B], F32, name="T64", tag="T64")
    nc.scalar.activation(out=T64, in_=mcum, func=AF.Exp, scale=-1.0)
    T64b = pre.tile([S, B], F32, name="T64b", tag="T64b")
    nc.scalar.activation(out=T64b, in_=vcum, func=AF.Exp, scale=-1.0)
    w64 = pre.tile([S, B], F32, name="w64", tag="w64")
    nc.vector.tensor_sub(w64, T64, T64b)

    # out_c = sum_s 0.5 * (w_s + w_s * tanh((TH_cs + cv_c)/2))
    for c in range(3):
        P8 = pre.tile([S, B], F32, name=f"P8{c}", tag=f"P8{c}")
        nc.gpsimd.tensor_add(P8, DTH[:, 1 + c, :], CVB[c])
        THt = pre.tile([S, B], F32, name=f"THt{c}", tag=f"THt{c}")
        nc.scalar.activation(out=THt, in_=P8, func=AF.Tanh, scale=0.5)
        P = pre.tile([S, B], BF16, name=f"P{c}", tag=f"P{c}")
        nc.vector.tensor_mul(P, w64, THt)
        nc.vector.tensor_add(P, P, w64)
        pc = psA.tile([128, B], F32, name="mmA", tag="mmA")[0:1, :]
        nc.tensor.matmul(pc, sb["half641"], P, start=True, stop=True)
        oc = pre.tile([1, B], F32, name=f"oc{c}", tag=f"oc{c}")
        nc.scalar.activation(out=oc, in_=pc, func=AF.Copy)
        nc.sync.dma_start(out=out_ap.rearrange("b c -> c b")[c : c + 1], in_=oc)


def build_nc():
    nc = bacc.Bacc("TRN2", target_bir_lowering=False, debug=False)
    specs = input_specs()
    aps = {
        name: nc.dram_tensor(name, list(shape), dt, kind="ExternalInput").ap()
        for name, (shape, dt) in specs.items()
    }
    out = nc.dram_tensor("out", [BL, 3], F32, kind="ExternalOutput").ap()
    with tile.TileContext(nc) as tc, ExitStack() as ctx:
        build_nerf(tc, ctx, out, aps)
    nc.compile()
    return nc


def make_in_maps(inputs):
    consts = host_constants()
    wts = host_weights(inputs)
    shared = {**consts, **wts}
    specs = input_specs()
    np_dt = {F32: np.float32, BF16: NP_BF16, F8: NP_F8}
    shared = {
        k: np.ascontiguousarray(np.asarray(v).astype(np_dt[specs[k][1]]))
        for k, v in shared.items()
    }
    in_maps = []
    for core in range(N_CORES):
        sl = slice(core * BL, (core + 1) * BL)
        m = dict(shared)
        m["xT"] = np.ascontiguousarray(np.asarray(inputs["x"])[sl].T, dtype=np.float32)
        m["off"] = np.ascontiguousarray(
            np.asarray(inputs["offsets"])[:, sl], dtype=np.float32
        )
        in_maps.append(m)
    return in_maps


def kernel(**inputs):
    from concourse.bass_utils import run_bass_kernel_spmd

    nc = build_nc()
    in_maps = make_in_maps(inputs)
    res = run_bass_kernel_spmd(nc, in_maps, core_ids=list(range(N_CORES)))
    out = np.concatenate([r["out"] for r in res.results], axis=0)
    return out.astype(np.float32)
